# revision 14
# baseline (speedup 1.0000x reference)
"""Causal multi-head attention (B=2, S=2048, D=1024, 16 heads of 64) on 8 TRN2
NeuronCores.

Sharding: core c -> batch b = c//4, head-group g = c%4 (4 heads = 256 model
dims per core).  Wq/Wk/Wv column-parallel, Wo row-parallel; the 4 partial
outputs per batch are summed on the host (no collectives).

v4: TensorE streams ~106us of matmul columns, ScalarE ~100us of softmax exp;
attention blocks (asc. j) are scalar-bound, so projection/Wo work is chopped
into PSUM-group "atoms" and interleaved into the attention slots.  Per slot
the issue order is [AV(ki-1), fillers, scores(ki)+exp] so the score matmul
(which waits on the st-buffer WAR vs the previous exp) is issued
just-in-time and never head-of-line-blocks the fillers.  PSUM (8 banks):
st 2x[128,512] per-head score tiles, av 3x[65,512] (K=128 AV with ones-row
-> softmax denom in row 64), two filler pools (2+1 banks, atoms alternate).
All wo evacs on DVE; the last Wo m-group is t-split so its t0 half overlaps
the final attention block and its t1+evac (ScalarE, idle by then) is the
only tail.  Host: out[b] = sum of 4 head-group partials (+ tail split) + bo.
"""

import numpy as np
import ml_dtypes

B, S, D = 2, 2048, 1024
HD = 64
NH = D // HD
N_CORES = 8
GROUPS = 4          # head-groups (tensor-parallel)
JG = D // GROUPS    # local dims per core = 256
NHL = JG // HD      # local heads = 4
KCH = D // 128      # contraction chunks for projections = 8
NKT = S // 128      # sk tiles = 16
NSB = S // 512      # 512-col blocks = 4

BF16 = ml_dtypes.bfloat16

_cached = {}


def _build():
    import concourse.bacc as bacc
    import concourse.tile as tile
    import concourse.mybir as mybir

    f32 = mybir.dt.float32
    bf16 = mybir.dt.bfloat16
    Exp = mybir.ActivationFunctionType.Exp

    nc = bacc.Bacc("TRN2", target_bir_lowering=False, debug=False,
                   num_devices=N_CORES)

    xTb = nc.dram_tensor("xTb", [NSB, 128, KCH, 512], bf16,
                         kind="ExternalInput").ap()
    wqT = nc.dram_tensor("wqT", [128, KCH, JG], bf16, kind="ExternalInput").ap()
    wkT = nc.dram_tensor("wkT", [128, KCH, JG], bf16, kind="ExternalInput").ap()
    wvT = nc.dram_tensor("wvT", [128, KCH, JG], bf16, kind="ExternalInput").ap()
    woT = nc.dram_tensor("woT", [128, 2, D], bf16, kind="ExternalInput").ap()
    bqc = nc.dram_tensor("bqc", [JG, 1], f32, kind="ExternalInput").ap()
    bkc = nc.dram_tensor("bkc", [JG, 1], f32, kind="ExternalInput").ap()
    bvb = nc.dram_tensor("bvb", [128, JG], f32, kind="ExternalInput").ap()
    maskT = nc.dram_tensor("maskT", [128, 128], bf16, kind="ExternalInput").ap()
    # rows 0..1535 of the output; the last 512 rows ship as two t-partials
    out = nc.dram_tensor("out", [S - 512, D], bf16, kind="ExternalOutput").ap()
    outx = nc.dram_tensor("outx", [2, 512, D], bf16, kind="ExternalOutput").ap()

    with tile.TileContext(nc) as tc:
        with (
            tc.tile_pool(name="const", bufs=1) as cpool,
            tc.tile_pool(name="pbig", bufs=2) as p_pool,
            tc.tile_pool(name="small", bufs=4) as small_pool,
            tc.tile_pool(name="outp", bufs=3) as out_pool,
            tc.tile_pool(name="st_ps", bufs=2, space="PSUM") as st_ps,
            tc.tile_pool(name="av_ps", bufs=3, space="PSUM") as av_ps,
            tc.tile_pool(name="pja", bufs=1, space="PSUM") as pja_ps,
            tc.tile_pool(name="pjb", bufs=1, space="PSUM") as pjb_ps,
        ):
            # ---- DMA staging: first-use order so PE starts at ~1.5us ----
            wq_sb = cpool.tile([128, KCH, JG], bf16)
            nc.sync.dma_start(wq_sb[:], wqT[:])
            bq_sb = cpool.tile([128, 2], f32)
            nc.sync.dma_start(bq_sb[:], bqc.rearrange("(t p) o -> p (t o)", p=128))
            xt_all = cpool.tile([128, KCH, S], bf16)
            nc.sync.dma_start(xt_all[:, :, 0:512], xTb[0])
            wk_sb = cpool.tile([128, KCH, JG], bf16)
            nc.sync.dma_start(wk_sb[:], wkT[:])
            bk_sb = cpool.tile([128, 2], f32)
            nc.sync.dma_start(bk_sb[:], bkc.rearrange("(t p) o -> p (t o)", p=128))
            wv_sb = cpool.tile([128, KCH, JG], bf16)
            nc.sync.dma_start(wv_sb[:], wvT[:])
            bvb_sb = cpool.tile([128, JG], f32)
            nc.sync.dma_start(bvb_sb[:], bvb[:])
            mask_sb = cpool.tile([128, 128], bf16)
            nc.sync.dma_start(mask_sb[:], maskT[:])
            for b in range(1, NSB):
                nc.sync.dma_start(xt_all[:, :, 512 * b:512 * b + 512], xTb[b])
            wo_sb = cpool.tile([128, 2, D], bf16)
            nc.sync.dma_start(wo_sb[:], woT[:])

            qt = [cpool.tile([128, S], bf16, name=f"qt{t}") for t in range(2)]
            kt = [cpool.tile([128, S], bf16, name=f"kt{t}") for t in range(2)]
            v_all = cpool.tile([128, NKT, NHL * 65], bf16)
            nc.vector.memset(
                v_all.rearrange("p k (h c) -> p k h c", c=65)[:, :, :, 64:65], 1.0)
            po = [cpool.tile([128, S], bf16, name=f"po{t}") for t in range(2)]
            # prime the exp table load (~2.7us) before the first real exp
            warm = small_pool.tile([1, 4], f32, tag="r1")
            nc.vector.memset(warm[:], 0.0)
            nc.scalar.activation(warm[:], warm[:], Exp)

            # ---- filler atoms: one short PSUM group each; alternate pools ----
            psel = [0]

            def pj_tile(name):
                psel[0] ^= 1
                if psel[0]:
                    return pja_ps.tile([128, 1024], f32, tag="pja", name=name)
                return pjb_ps.tile([128, 512], f32, tag="pjb", name=name)

            def qk_atom(w_sb, b_sb, dst, t, blk):
                ps = pj_tile(f"qk{t}_{blk}")
                for k in range(KCH):
                    nc.tensor.matmul(
                        ps[:, 0:512],
                        lhsT=w_sb[:, k, 128 * t:128 * t + 128],
                        rhs=xt_all[:, k, 512 * blk:512 * blk + 512],
                        start=(k == 0), stop=(k == KCH - 1))
                nc.vector.tensor_scalar_add(
                    dst[t][:, 512 * blk:512 * blk + 512], ps[:, 0:512],
                    b_sb[:, t:t + 1])

            def v_atom(si):
                ps = pj_tile(f"v{si}")
                for k in range(KCH):
                    nc.tensor.matmul(
                        ps[:, 0:256],
                        lhsT=xt_all[:, k, 128 * si:128 * si + 128],
                        rhs=wv_sb[:, k, :],
                        start=(k == 0), stop=(k == KCH - 1))
                nc.vector.tensor_add(
                    v_all[:, si, :].rearrange("p (h c) -> p h c", c=65)[:, :, 0:64],
                    ps[:, 0:256].rearrange("p (h c) -> p h c", c=64),
                    bvb_sb.rearrange("p (h c) -> p h c", c=64))

            # wo m-tile: 4 accumulating MMs, one [128,1024] DVE evac, one DMA
            def wo_atom(m):
                ps = pja_ps.tile([128, 1024], f32, tag="pja", name=f"wo{m}")
                psel[0] = 1
                for t in range(2):
                    for nh2 in range(2):
                        nc.tensor.matmul(
                            ps[:, 512 * nh2:512 * nh2 + 512],
                            lhsT=po[t][:, 128 * m:128 * m + 128],
                            rhs=wo_sb[:, t, 512 * nh2:512 * nh2 + 512],
                            start=(t == 0), stop=(t == 1))
                ob = out_pool.tile([128, 1024], bf16, tag="ob", name=f"ob{m}")
                nc.vector.tensor_copy(ob[:], ps[:])
                nc.sync.dma_start(out[128 * m:128 * m + 128, :], ob[:])

            # wo for m-tiles 12..15: t-split so t0 runs during attn(1,3)
            def wox_atom(m, t):
                ps = pja_ps.tile([128, 1024], f32, tag="pja",
                                 name=f"wox{m}_{t}")
                psel[0] = 1
                for nh2 in range(2):
                    nc.tensor.matmul(
                        ps[:, 512 * nh2:512 * nh2 + 512],
                        lhsT=po[t][:, 128 * m:128 * m + 128],
                        rhs=wo_sb[:, t, 512 * nh2:512 * nh2 + 512],
                        start=True, stop=True)
                ob = out_pool.tile([128, 1024], bf16, tag="ob",
                                   name=f"obx{m}_{t}")
                if t == 0:
                    nc.vector.tensor_copy(ob[:], ps[:])
                else:
                    nc.scalar.copy(ob[:], ps[:])
                nc.sync.dma_start(outx[t, 128 * (m - 12):128 * (m - 12) + 128, :],
                                  ob[:])

            # ---- attention block: pair of heads x one 512-col query block ----
            def attn_block(pair, j, fillers):
                nk = 4 * (j + 1)
                qt_t, kt_t = qt[pair], kt[pair]
                pt = p_pool.tile([128, NKT, 1024], bf16, tag="p",
                                 name=f"pt{pair}_{j}")
                avp = [av_ps.tile([65, 512], f32, tag="av",
                                  name=f"av{pair}_{j}_{hh}") for hh in range(2)]
                nf = len(fillers)
                fi = 0
                for ki in range(nk + 1):
                    # AV for the previous tile first (its exp is done by now)
                    if ki >= 1:
                        ka = ki - 1
                        da = max(0, 128 * ka - 512 * j)
                        for hh in range(2):
                            h = 2 * pair + hh
                            nc.tensor.matmul(
                                avp[hh][0:65, da:512],
                                lhsT=v_all[:, ka, 65 * h:65 * h + 65],
                                rhs=pt[:, ka, 512 * hh + da:512 * hh + 512],
                                start=(ka == 0), stop=(ka == nk - 1))
                    while fi * (nk + 1) < nf * (ki + 1):
                        fillers[fi]()
                        fi += 1
                    # scores issued just-in-time: their st-buffer WAR wait
                    # (vs exp ki-2) must not head-of-line-block the fillers
                    if ki < nk:
                        d = max(0, 128 * ki - 512 * j)
                        for hh in range(2):
                            sth = st_ps.tile([128, 512], f32, tag="st",
                                             name=f"st{pair}_{j}_{ki}_{hh}")
                            nc.tensor.matmul(
                                sth[:, d:512],
                                lhsT=kt_t[64 * hh:64 * hh + 64,
                                          128 * ki:128 * ki + 128],
                                rhs=qt_t[64 * hh:64 * hh + 64,
                                         512 * j + d:512 * j + 512],
                                start=True, stop=True)
                            nc.scalar.activation(
                                pt[:, ki, 512 * hh + d:512 * hh + 512],
                                sth[:, d:512], Exp)
                            if ki >= 4 * j:
                                # causal diag tile: zero where sq < sk
                                nc.vector.tensor_mul(
                                    pt[:, ki, 512 * hh + d:512 * hh + d + 128],
                                    pt[:, ki, 512 * hh + d:512 * hh + d + 128],
                                    mask_sb[:])
                # evac: preoutT = avp[0:64] * (1 / avp[64]) -> po bf16
                for hh in range(2):
                    lrow = small_pool.tile([1, 512], f32, tag="r1")
                    nc.vector.tensor_copy(lrow[:], avp[hh][64:65, :])
                    rbr = small_pool.tile([1, 512], f32, tag="r1")
                    # custom-DVE ops mis-read PSUM on HW: recip from SBUF only
                    nc.vector.reciprocal_approx_fast(rbr[:], lrow[:])
                    rb = small_pool.tile([64, 512], f32, tag="r64")
                    nc.gpsimd.partition_broadcast(rb[:], rbr[:])
                    nc.vector.tensor_mul(
                        po[pair][64 * hh:64 * hh + 64, 512 * j:512 * j + 512],
                        avp[hh][0:64, :], rb[:])

            # ---- schedule ----
            qk_atom(wq_sb, bq_sb, qt, 0, 0)
            qk_atom(wk_sb, bk_sb, kt, 0, 0)

            attn_block(0, 0, [
                lambda: v_atom(0), lambda: v_atom(1),
                lambda: v_atom(2), lambda: v_atom(3),
                lambda: qk_atom(wq_sb, bq_sb, qt, 0, 1),
                lambda: qk_atom(wk_sb, bk_sb, kt, 0, 1),
            ])
            attn_block(0, 1, [
                lambda: v_atom(4), lambda: v_atom(5),
                lambda: v_atom(6), lambda: v_atom(7),
                lambda: qk_atom(wq_sb, bq_sb, qt, 1, 0),
                lambda: qk_atom(wk_sb, bk_sb, kt, 1, 0),
            ])
            attn_block(1, 0, [
                lambda: qk_atom(wq_sb, bq_sb, qt, 1, 1),
                lambda: qk_atom(wk_sb, bk_sb, kt, 1, 1),
            ])
            attn_block(1, 1, [
                lambda: wo_atom(0), lambda: wo_atom(1),
                lambda: qk_atom(wq_sb, bq_sb, qt, 0, 2),
                lambda: qk_atom(wk_sb, bk_sb, kt, 0, 2),
            ])
            attn_block(0, 2, [
                lambda: v_atom(8), lambda: v_atom(9),
                lambda: v_atom(10), lambda: v_atom(11),
                lambda: qk_atom(wq_sb, bq_sb, qt, 1, 2),
                lambda: qk_atom(wk_sb, bk_sb, kt, 1, 2),
                lambda: wo_atom(2), lambda: wo_atom(3),
            ])
            attn_block(1, 2, [
                lambda: qk_atom(wq_sb, bq_sb, qt, 0, 3),
                lambda: qk_atom(wk_sb, bk_sb, kt, 0, 3),
                lambda: wo_atom(4), lambda: wo_atom(5),
                lambda: wo_atom(6),
            ])
            attn_block(0, 3, [
                lambda: v_atom(12), lambda: v_atom(13),
                lambda: v_atom(14), lambda: v_atom(15),
                lambda: qk_atom(wq_sb, bq_sb, qt, 1, 3),
                lambda: qk_atom(wk_sb, bk_sb, kt, 1, 3),
                lambda: wo_atom(7), lambda: wo_atom(8),
            ])
            attn_block(1, 3, [
                lambda: wo_atom(9), lambda: wo_atom(10),
                lambda: wo_atom(11),
                lambda: wox_atom(12, 0), lambda: wox_atom(13, 0),
                lambda: wox_atom(14, 0), lambda: wox_atom(15, 0),
            ])
            for m in range(12, 16):
                wox_atom(m, 1)

    nc.compile()
    return nc


def _get_nc():
    if "nc" not in _cached:
        _cached["nc"] = _build()
    return _cached["nc"]


def _make_in_maps(x, Wq, bq, Wk, bk, Wv, bv, Wo):
    sc = 1.0 / np.sqrt(HD)
    tri = np.arange(128)
    mask = np.where(tri[:, None] <= tri[None, :], 1.0, 0.0).astype(BF16)
    in_maps = []
    for c in range(N_CORES):
        b, g = divmod(c, GROUPS)
        sl = slice(JG * g, JG * (g + 1))
        def tile_k(a):  # [D, M] -> [128, D//128, M] contiguous
            return np.ascontiguousarray(
                a.reshape(a.shape[0] // 128, 128, a.shape[1]).transpose(1, 0, 2))

        xt = tile_k(x[b].T.astype(BF16))  # [128, KCH, S]
        xtb = np.ascontiguousarray(
            xt.reshape(128, KCH, NSB, 512).transpose(2, 0, 1, 3))
        in_maps.append({
            "maskT": mask,
            "xTb": xtb,
            "wqT": tile_k((Wq[sl] * sc).T.astype(BF16)),
            "wkT": tile_k(Wk[sl].T.astype(BF16)),
            "wvT": tile_k(Wv[sl].T.astype(BF16)),
            "woT": tile_k(Wo[:, sl].T.astype(BF16)),
            "bqc": (bq[sl] * sc).astype(np.float32).reshape(JG, 1),
            "bkc": bk[sl].astype(np.float32).reshape(JG, 1),
            "bvb": np.broadcast_to(bv[sl].astype(np.float32), (128, JG)).copy(),
        })
    return in_maps


def _assemble(results, bo):
    full = np.empty((B, S, D), np.float32)
    for b in range(B):
        acc = np.empty((S, D), np.float32)
        r0 = results[4 * b]
        acc[0:S - 512] = r0["out"]
        acc[S - 512:] = r0["outx"][0].astype(np.float32) + r0["outx"][1]
        for g in range(1, GROUPS):
            r = results[4 * b + g]
            acc[0:S - 512] += r["out"]
            acc[S - 512:] += r["outx"][0].astype(np.float32) + r["outx"][1]
        full[b] = acc + np.asarray(bo, np.float32)[None, :]
    return full


def kernel(x, Wq, bq, Wk, bk, Wv, bv, Wo, bo, _return_results=False):
    from concourse.bass_utils import run_bass_kernel_spmd

    nc = _get_nc()
    in_maps = _make_in_maps(np.asarray(x, np.float32), np.asarray(Wq, np.float32),
                            np.asarray(bq, np.float32), np.asarray(Wk, np.float32),
                            np.asarray(bk, np.float32), np.asarray(Wv, np.float32),
                            np.asarray(bv, np.float32), np.asarray(Wo, np.float32))
    res = run_bass_kernel_spmd(nc, in_maps, core_ids=list(range(N_CORES)))
    full = _assemble(res.results, bo)
    if _return_results:
        return full, res
    return full


# revision 16
# speedup vs baseline: 1.2661x; 1.2661x over previous
"""Causal multi-head attention (B=2, S=2048, D=1024, 16 heads of 64) on 8 TRN2
NeuronCores.

Sharding: core c -> batch b = c//4, head-group g = c%4 (4 heads = 256 model
dims per core).  Wq/Wk/Wv column-parallel, Wo row-parallel; the 4 partial
outputs per batch are summed on the host (no collectives).

v4: TensorE streams ~106us of matmul columns, ScalarE ~100us of softmax exp;
attention blocks (asc. j) are scalar-bound, so projection/Wo work is chopped
into PSUM-group "atoms" and interleaved into the attention slots.  Per slot
the issue order is [AV(ki-1), fillers, scores(ki)+exp] so the score matmul
(which waits on the st-buffer WAR vs the previous exp) is issued
just-in-time and never head-of-line-blocks the fillers.  PSUM (8 banks):
st 2x[128,512] per-head score tiles, av 3x[65,512] (K=128 AV with ones-row
-> softmax denom in row 64), two filler pools (2+1 banks, atoms alternate).
All wo evacs on DVE; the last Wo m-group is t-split so its t0 half overlaps
the final attention block and its t1+evac (ScalarE, idle by then) is the
only tail.  Host: out[b] = sum of 4 head-group partials (+ tail split) + bo.
"""

import numpy as np
import ml_dtypes

B, S, D = 2, 2048, 1024
HD = 64
NH = D // HD
N_CORES = 8
GROUPS = 4          # head-groups (tensor-parallel)
JG = D // GROUPS    # local dims per core = 256
NHL = JG // HD      # local heads = 4
KCH = D // 128      # contraction chunks for projections = 8
NKT = S // 128      # sk tiles = 16
NSB = S // 512      # 512-col blocks = 4

BF16 = ml_dtypes.bfloat16

_cached = {}


def _build():
    import concourse.bacc as bacc
    import concourse.tile as tile
    import concourse.mybir as mybir

    f32 = mybir.dt.float32
    bf16 = mybir.dt.bfloat16
    Exp = mybir.ActivationFunctionType.Exp

    nc = bacc.Bacc("TRN2", target_bir_lowering=False, debug=False,
                   num_devices=N_CORES)

    xTb = nc.dram_tensor("xTb", [NSB, 128, KCH, 512], bf16,
                         kind="ExternalInput").ap()
    wqT = nc.dram_tensor("wqT", [128, KCH, JG], bf16, kind="ExternalInput").ap()
    wkT = nc.dram_tensor("wkT", [128, KCH, JG], bf16, kind="ExternalInput").ap()
    wvT = nc.dram_tensor("wvT", [128, KCH, JG], bf16, kind="ExternalInput").ap()
    woT = nc.dram_tensor("woT", [128, 2, D], bf16, kind="ExternalInput").ap()
    bqc = nc.dram_tensor("bqc", [JG, 1], f32, kind="ExternalInput").ap()
    bkc = nc.dram_tensor("bkc", [JG, 1], f32, kind="ExternalInput").ap()
    bvb = nc.dram_tensor("bvb", [128, JG], f32, kind="ExternalInput").ap()
    maskT = nc.dram_tensor("maskT", [128, 128], bf16, kind="ExternalInput").ap()
    # rows 0..1535 of the output; the last 512 rows ship as two t-partials
    out = nc.dram_tensor("out", [S - 512, D], bf16, kind="ExternalOutput").ap()
    outx = nc.dram_tensor("outx", [2, 512, D], bf16, kind="ExternalOutput").ap()

    with tile.TileContext(nc) as tc:
        with (
            tc.tile_pool(name="const", bufs=1) as cpool,
            tc.tile_pool(name="pbig", bufs=2) as p_pool,
            tc.tile_pool(name="small", bufs=4) as small_pool,
            tc.tile_pool(name="outp", bufs=3) as out_pool,
            tc.tile_pool(name="st_ps", bufs=3, space="PSUM") as st_ps,
            tc.tile_pool(name="av_ps", bufs=2, space="PSUM") as av_ps,
            tc.tile_pool(name="pja", bufs=1, space="PSUM") as pja_ps,
            tc.tile_pool(name="pjb", bufs=1, space="PSUM") as pjb_ps,
        ):
            # ---- DMA staging: first-use order so PE starts at ~1.5us ----
            wq_sb = cpool.tile([128, KCH, JG], bf16)
            nc.sync.dma_start(wq_sb[:], wqT[:])
            bq_sb = cpool.tile([128, 2], f32)
            nc.sync.dma_start(bq_sb[:], bqc.rearrange("(t p) o -> p (t o)", p=128))
            xt_all = cpool.tile([128, KCH, S], bf16)
            nc.sync.dma_start(xt_all[:, :, 0:512], xTb[0])
            wk_sb = cpool.tile([128, KCH, JG], bf16)
            nc.sync.dma_start(wk_sb[:], wkT[:])
            bk_sb = cpool.tile([128, 2], f32)
            nc.sync.dma_start(bk_sb[:], bkc.rearrange("(t p) o -> p (t o)", p=128))
            wv_sb = cpool.tile([128, KCH, JG], bf16)
            nc.sync.dma_start(wv_sb[:], wvT[:])
            bvb_sb = cpool.tile([128, JG], f32)
            nc.sync.dma_start(bvb_sb[:], bvb[:])
            mask_sb = cpool.tile([128, 128], bf16)
            nc.sync.dma_start(mask_sb[:], maskT[:])
            for b in range(1, NSB):
                nc.sync.dma_start(xt_all[:, :, 512 * b:512 * b + 512], xTb[b])
            wo_sb = cpool.tile([128, 2, D], bf16)
            nc.sync.dma_start(wo_sb[:], woT[:])

            qt = [cpool.tile([128, S], bf16, name=f"qt{t}") for t in range(2)]
            kt = [cpool.tile([128, S], bf16, name=f"kt{t}") for t in range(2)]
            v_all = cpool.tile([128, NKT, NHL * 65], bf16)
            nc.vector.memset(
                v_all.rearrange("p k (h c) -> p k h c", c=65)[:, :, :, 64:65], 1.0)
            po = [cpool.tile([128, S], bf16, name=f"po{t}") for t in range(2)]
            # prime the exp table load (~2.7us) before the first real exp
            warm = small_pool.tile([1, 4], f32, tag="r1")
            nc.vector.memset(warm[:], 0.0)
            nc.scalar.activation(warm[:], warm[:], Exp)

            # ---- filler atoms: one short PSUM group each; alternate pools ----
            psel = [0]

            def pj_tile(name):
                psel[0] ^= 1
                if psel[0]:
                    return pja_ps.tile([128, 1024], f32, tag="pja", name=name)
                return pjb_ps.tile([128, 512], f32, tag="pjb", name=name)

            def qk_atom(w_sb, b_sb, dst, t, blk):
                ps = pj_tile(f"qk{t}_{blk}")
                for k in range(KCH):
                    nc.tensor.matmul(
                        ps[:, 0:512],
                        lhsT=w_sb[:, k, 128 * t:128 * t + 128],
                        rhs=xt_all[:, k, 512 * blk:512 * blk + 512],
                        start=(k == 0), stop=(k == KCH - 1))
                nc.vector.tensor_scalar_add(
                    dst[t][:, 512 * blk:512 * blk + 512], ps[:, 0:512],
                    b_sb[:, t:t + 1])

            def v_atom(si):
                ps = pj_tile(f"v{si}")
                for k in range(KCH):
                    nc.tensor.matmul(
                        ps[:, 0:256],
                        lhsT=xt_all[:, k, 128 * si:128 * si + 128],
                        rhs=wv_sb[:, k, :],
                        start=(k == 0), stop=(k == KCH - 1))
                nc.vector.tensor_add(
                    v_all[:, si, :].rearrange("p (h c) -> p h c", c=65)[:, :, 0:64],
                    ps[:, 0:256].rearrange("p (h c) -> p h c", c=64),
                    bvb_sb.rearrange("p (h c) -> p h c", c=64))

            # wo m-tile: 4 accumulating MMs, one [128,1024] DVE evac, one DMA
            def wo_atom(m):
                ps = pja_ps.tile([128, 1024], f32, tag="pja", name=f"wo{m}")
                psel[0] = 1
                for t in range(2):
                    for nh2 in range(2):
                        nc.tensor.matmul(
                            ps[:, 512 * nh2:512 * nh2 + 512],
                            lhsT=po[t][:, 128 * m:128 * m + 128],
                            rhs=wo_sb[:, t, 512 * nh2:512 * nh2 + 512],
                            start=(t == 0), stop=(t == 1))
                ob = out_pool.tile([128, 1024], bf16, tag="ob", name=f"ob{m}")
                nc.vector.tensor_copy(ob[:], ps[:])
                nc.sync.dma_start(out[128 * m:128 * m + 128, :], ob[:])

            # wo for m-tiles 12..15: t-split so t0 runs during attn(1,3)
            def wox_atom(m, t):
                ps = pja_ps.tile([128, 1024], f32, tag="pja",
                                 name=f"wox{m}_{t}")
                psel[0] = 1
                for nh2 in range(2):
                    nc.tensor.matmul(
                        ps[:, 512 * nh2:512 * nh2 + 512],
                        lhsT=po[t][:, 128 * m:128 * m + 128],
                        rhs=wo_sb[:, t, 512 * nh2:512 * nh2 + 512],
                        start=True, stop=True)
                ob = out_pool.tile([128, 1024], bf16, tag="ob",
                                   name=f"obx{m}_{t}")
                if t == 0:
                    nc.vector.tensor_copy(ob[:], ps[:])
                else:
                    nc.scalar.copy(ob[:], ps[:])
                nc.sync.dma_start(outx[t, 128 * (m - 12):128 * (m - 12) + 128, :],
                                  ob[:])

            # ---- attention block: pair of heads x one 512-col query block ----
            def attn_block(pair, j, fillers):
                nk = 4 * (j + 1)
                qt_t, kt_t = qt[pair], kt[pair]
                pt = p_pool.tile([128, NKT, 1024], bf16, tag="p",
                                 name=f"pt{pair}_{j}")
                avp = [av_ps.tile([65, 512], f32, tag="av",
                                  name=f"av{pair}_{j}_{hh}") for hh in range(2)]
                nf = len(fillers)
                fi = 0
                for ki in range(nk + 2):
                    # scores first: with st bufs=3 their st-buffer WAR (vs
                    # exp ki-2) is already clear, so the exp pacer never waits
                    if ki < nk:
                        d = max(0, 128 * ki - 512 * j)
                        for hh in range(2):
                            sth = st_ps.tile([128, 512], f32, tag="st",
                                             name=f"st{pair}_{j}_{ki}_{hh}")
                            nc.tensor.matmul(
                                sth[:, d:512],
                                lhsT=kt_t[64 * hh:64 * hh + 64,
                                          128 * ki:128 * ki + 128],
                                rhs=qt_t[64 * hh:64 * hh + 64,
                                         512 * j + d:512 * j + 512],
                                start=True, stop=True)
                            nc.scalar.activation(
                                pt[:, ki, 512 * hh + d:512 * hh + 512],
                                sth[:, d:512], Exp)
                            if ki >= 4 * j:
                                # causal diag tile: zero where sq < sk
                                nc.vector.tensor_mul(
                                    pt[:, ki, 512 * hh + d:512 * hh + d + 128],
                                    pt[:, ki, 512 * hh + d:512 * hh + d + 128],
                                    mask_sb[:])
                    # AV lags 2 slots: its exp is long done, and at block
                    # start the av-buffer WAR (prev block's evac) has cleared
                    if ki >= 2:
                        ka = ki - 2
                        da = max(0, 128 * ka - 512 * j)
                        for hh in range(2):
                            h = 2 * pair + hh
                            nc.tensor.matmul(
                                avp[hh][0:65, da:512],
                                lhsT=v_all[:, ka, 65 * h:65 * h + 65],
                                rhs=pt[:, ka, 512 * hh + da:512 * hh + 512],
                                start=(ka == 0), stop=(ka == nk - 1))
                    while fi * (nk + 2) < nf * (ki + 1):
                        fillers[fi]()
                        fi += 1
                # evac: preoutT = avp[0:64] * (1 / avp[64]) -> po bf16
                for hh in range(2):
                    lrow = small_pool.tile([1, 512], f32, tag="r1")
                    nc.vector.tensor_copy(lrow[:], avp[hh][64:65, :])
                    rbr = small_pool.tile([1, 512], f32, tag="r1")
                    # custom-DVE ops mis-read PSUM on HW: recip from SBUF only
                    nc.vector.reciprocal_approx_fast(rbr[:], lrow[:])
                    rb = small_pool.tile([64, 512], f32, tag="r64")
                    nc.gpsimd.partition_broadcast(rb[:], rbr[:])
                    nc.vector.tensor_mul(
                        po[pair][64 * hh:64 * hh + 64, 512 * j:512 * j + 512],
                        avp[hh][0:64, :], rb[:])

            # ---- schedule ----
            qk_atom(wq_sb, bq_sb, qt, 0, 0)
            qk_atom(wk_sb, bk_sb, kt, 0, 0)

            attn_block(0, 0, [
                lambda: v_atom(0), lambda: v_atom(1),
                lambda: v_atom(2), lambda: v_atom(3),
                lambda: qk_atom(wq_sb, bq_sb, qt, 0, 1),
                lambda: qk_atom(wk_sb, bk_sb, kt, 0, 1),
            ])
            attn_block(0, 1, [
                lambda: v_atom(4), lambda: v_atom(5),
                lambda: v_atom(6), lambda: v_atom(7),
                lambda: qk_atom(wq_sb, bq_sb, qt, 1, 0),
                lambda: qk_atom(wk_sb, bk_sb, kt, 1, 0),
            ])
            attn_block(1, 0, [
                lambda: qk_atom(wq_sb, bq_sb, qt, 1, 1),
                lambda: qk_atom(wk_sb, bk_sb, kt, 1, 1),
            ])
            attn_block(1, 1, [
                lambda: wo_atom(0), lambda: wo_atom(1),
                lambda: qk_atom(wq_sb, bq_sb, qt, 0, 2),
                lambda: qk_atom(wk_sb, bk_sb, kt, 0, 2),
            ])
            attn_block(0, 2, [
                lambda: v_atom(8), lambda: v_atom(9),
                lambda: v_atom(10), lambda: v_atom(11),
                lambda: qk_atom(wq_sb, bq_sb, qt, 1, 2),
                lambda: qk_atom(wk_sb, bk_sb, kt, 1, 2),
                lambda: wo_atom(2), lambda: wo_atom(3),
            ])
            attn_block(1, 2, [
                lambda: qk_atom(wq_sb, bq_sb, qt, 0, 3),
                lambda: qk_atom(wk_sb, bk_sb, kt, 0, 3),
                lambda: wo_atom(4), lambda: wo_atom(5),
                lambda: wo_atom(6),
            ])
            attn_block(0, 3, [
                lambda: v_atom(12), lambda: v_atom(13),
                lambda: v_atom(14), lambda: v_atom(15),
                lambda: qk_atom(wq_sb, bq_sb, qt, 1, 3),
                lambda: qk_atom(wk_sb, bk_sb, kt, 1, 3),
                lambda: wo_atom(7), lambda: wo_atom(8),
            ])
            attn_block(1, 3, [
                lambda: wo_atom(9), lambda: wo_atom(10),
                lambda: wo_atom(11),
                lambda: wox_atom(12, 0), lambda: wox_atom(13, 0),
                lambda: wox_atom(14, 0), lambda: wox_atom(15, 0),
            ])
            for m in range(12, 16):
                wox_atom(m, 1)

    nc.compile()
    return nc


def _get_nc():
    if "nc" not in _cached:
        _cached["nc"] = _build()
    return _cached["nc"]


def _make_in_maps(x, Wq, bq, Wk, bk, Wv, bv, Wo):
    sc = 1.0 / np.sqrt(HD)
    tri = np.arange(128)
    mask = np.where(tri[:, None] <= tri[None, :], 1.0, 0.0).astype(BF16)
    in_maps = []
    for c in range(N_CORES):
        b, g = divmod(c, GROUPS)
        sl = slice(JG * g, JG * (g + 1))
        def tile_k(a):  # [D, M] -> [128, D//128, M] contiguous
            return np.ascontiguousarray(
                a.reshape(a.shape[0] // 128, 128, a.shape[1]).transpose(1, 0, 2))

        xt = tile_k(x[b].T.astype(BF16))  # [128, KCH, S]
        xtb = np.ascontiguousarray(
            xt.reshape(128, KCH, NSB, 512).transpose(2, 0, 1, 3))
        in_maps.append({
            "maskT": mask,
            "xTb": xtb,
            "wqT": tile_k((Wq[sl] * sc).T.astype(BF16)),
            "wkT": tile_k(Wk[sl].T.astype(BF16)),
            "wvT": tile_k(Wv[sl].T.astype(BF16)),
            "woT": tile_k(Wo[:, sl].T.astype(BF16)),
            "bqc": (bq[sl] * sc).astype(np.float32).reshape(JG, 1),
            "bkc": bk[sl].astype(np.float32).reshape(JG, 1),
            "bvb": np.broadcast_to(bv[sl].astype(np.float32), (128, JG)).copy(),
        })
    return in_maps


def _assemble(results, bo):
    full = np.empty((B, S, D), np.float32)
    for b in range(B):
        acc = np.empty((S, D), np.float32)
        r0 = results[4 * b]
        acc[0:S - 512] = r0["out"]
        acc[S - 512:] = r0["outx"][0].astype(np.float32) + r0["outx"][1]
        for g in range(1, GROUPS):
            r = results[4 * b + g]
            acc[0:S - 512] += r["out"]
            acc[S - 512:] += r["outx"][0].astype(np.float32) + r["outx"][1]
        full[b] = acc + np.asarray(bo, np.float32)[None, :]
    return full


def kernel(x, Wq, bq, Wk, bk, Wv, bv, Wo, bo, _return_results=False):
    from concourse.bass_utils import run_bass_kernel_spmd

    nc = _get_nc()
    in_maps = _make_in_maps(np.asarray(x, np.float32), np.asarray(Wq, np.float32),
                            np.asarray(bq, np.float32), np.asarray(Wk, np.float32),
                            np.asarray(bk, np.float32), np.asarray(Wv, np.float32),
                            np.asarray(bv, np.float32), np.asarray(Wo, np.float32))
    res = run_bass_kernel_spmd(nc, in_maps, core_ids=list(range(N_CORES)))
    full = _assemble(res.results, bo)
    if _return_results:
        return full, res
    return full
